# revision 1
# baseline (speedup 1.0000x reference)
"""Chebyshev atomic descriptor kernel for 8 Trainium2 NeuronCores.

Math (matches the jax reference exactly up to fp reassociation):
  radial: per edge e with distance d:  xr = 2(d-MIN)/(RAD-MIN)-1,
    P_c = T_c(xr)*fc_rad(d) via the Chebyshev recurrence applied to P
    directly, summed per atom (24 contiguous edges/atom), plain and
    typespin-weighted.
  angular: triplets are ALL pairs (j<k) of each atom's 24 edges, so
    sum_{j<k} T_m(u_j.u_k) w_j w_k = (S_m - D*T_m(1))/2 with
    S_0 = M0^2, S_1 = |M1|^2, sum c^2 ww = |M2|^2, sum c^3 ww = |M3|^2
    where Mk = sum_j w_j u_j^(x)k  (symmetric moment tensors; unique
    entries with multiplicity weights), D = sum_j w_j^2.
    T_2 = 2c^2-1, T_3 = 4c^3-3c  ->  linear combos of the above.
    |u|=1 gives trace identities that derive 4 of the 20 channels:
      M2zz = M0-M2xx-M2yy,  M3xzz = M1x-M3xxx-M3xyy,
      M3yzz = M1y-M3xxy-M3yyy,  M3zzz = M1z-M3xxz-M3yyz.
  The typespin-weighted angular sums use weights w*s (s=+-1); D is
  unchanged since s^2=1.

Sharding: atoms 0..49999 padded to 50176 = 8 cores x 128 partitions x 49
atoms; each partition row owns 49 atoms x 24 contiguous edges.
"""

import numpy as np

N_ATOMS = 50000
K = 24
RAD_ORDER = 10
ANG_ORDER = 3
RAD_CUT = 8.0
ANG_CUT = 6.5
MIN_CUT = 0.55
NCORES = 8
A_PART = 49                      # atoms per partition
NA_CORE = 128 * A_PART           # 6272 atoms per core
NPAD = NCORES * NA_CORE          # 50176
NRAD = RAD_ORDER + 1             # 11
NOUT = 2 * (NRAD + ANG_ORDER + 1)  # 30

_COMPILED = {}


def build_program(loop_n: int = 1, nocombine: bool = False):
    """Build + compile the per-core Bass program. loop_n>1 repeats the whole
    body in a hardware loop (for timing)."""
    import concourse.bacc as bacc
    import concourse.mybir as mybir
    from concourse.tile import TileContext

    f32 = mybir.dt.float32
    i32 = mybir.dt.int32
    ALU = mybir.AluOpType
    ACTF = mybir.ActivationFunctionType
    AX = mybir.AxisListType

    nc = bacc.Bacc("TRN2", target_bir_lowering=False)

    # activation() biases must come from the const-AP registry
    pi2 = float(np.pi / 2)
    _cst = nc.alloc_sbuf_tensor("const-float32-pi2", [128, 1], f32)
    nc.gpsimd.memset(_cst.ap(), pi2)
    nc.const_aps.aps[(f32, pi2)] = _cst.ap()
    nc.all_engine_barrier()

    d_dram = nc.dram_tensor("d", [128, A_PART, K], f32, kind="ExternalInput")
    u_dram = nc.dram_tensor("u", [128, A_PART, K, 3], f32, kind="ExternalInput")
    sp_dram = nc.dram_tensor("sp", [128, A_PART, K], i32, kind="ExternalInput")
    out_dram = nc.dram_tensor("out", [128, A_PART, NOUT], f32, kind="ExternalOutput")

    E3 = [128, A_PART, K]

    with TileContext(nc) as tc:
        with (
            tc.tile_pool(name="per", bufs=1) as per,      # persistent channels
            tc.tile_pool(name="rot", bufs=4) as rot,      # rotating scratch
            tc.tile_pool(name="cheb", bufs=3) as chebp,   # chebyshev chain
            tc.tile_pool(name="shared", bufs=1) as shared,  # A/B-phase shared
        ):
            # multiplicity weights for |M2|^2, |M3|^2 — broadcast along atoms
            # ch: 0=M0 1:4=M1  4:10=M2(xx,yy,zz,xy,xz,yz)
            # 10:20=M3(xxx,xxy,xxz,xyy,yyy,yyz,xyz,xzz,yzz,zzz)
            wvec = per.tile([128, 1, 20], f32, tag="wvec")
            nc.vector.memset(wvec[:, :, 0:7], 1.0)
            nc.vector.memset(wvec[:, :, 7:10], 2.0)
            nc.vector.memset(wvec[:, :, 10:11], 1.0)
            nc.vector.memset(wvec[:, :, 11:14], 3.0)
            nc.vector.memset(wvec[:, :, 14:15], 1.0)
            nc.vector.memset(wvec[:, :, 15:16], 3.0)
            nc.vector.memset(wvec[:, :, 16:17], 6.0)
            nc.vector.memset(wvec[:, :, 17:20], 3.0)
            nc.vector.memset(wvec[:, :, 19:20], 1.0)

            def body(_iv=None):
                # ---- loads (split for DMA-queue parallelism) ----
                d_t = per.tile(E3, f32, tag="d")
                u_t = per.tile([128, A_PART, K, 3], f32, tag="u")
                sp_t = per.tile(E3, i32, tag="sp")
                for lo, hi in ((0, 64), (64, 128)):
                    nc.sync.dma_start(out=d_t[lo:hi, :, :], in_=d_dram.ap()[lo:hi])
                    nc.sync.dma_start(out=sp_t[lo:hi, :, :], in_=sp_dram.ap()[lo:hi])
                for lo, hi in ((0, 32), (32, 64), (64, 96), (96, 128)):
                    nc.sync.dma_start(out=u_t[lo:hi], in_=u_dram.ap()[lo:hi])

                out_t = per.tile([128, A_PART, NOUT], f32, tag="out")

                # ---- per-edge scalars ----
                ts_f = per.tile(E3, f32, tag="tsf")
                nc.vector.tensor_scalar(
                    out=ts_f[:, :, :], in0=sp_t[:, :, :],
                    scalar1=2.0, scalar2=-1.0, op0=ALU.mult, op1=ALU.add)

                # xr2 = 2*xr
                ax = 2.0 / (RAD_CUT - MIN_CUT)
                bx = -MIN_CUT * ax - 1.0
                xr2 = per.tile(E3, f32, tag="xr2")
                nc.vector.tensor_scalar(
                    out=xr2[:, :, :], in0=d_t[:, :, :],
                    scalar1=2.0 * ax, scalar2=2.0 * bx, op0=ALU.mult, op1=ALU.add)

                # fc_rad = 0.5*cos(pi*d/RAD_CUT)+0.5 ; cos(x)=sin(pi/2-x)
                c_r = per.tile(E3, f32, tag="c_r")
                nc.scalar.activation(
                    out=c_r[:, :, :], in_=d_t[:, :, :], func=ACTF.Sin,
                    bias=pi2, scale=float(-np.pi / RAD_CUT))

                # fc_ang = relu(cos(pi*d/(2*ANG_CUT)))^2  (exact 0 past cut)
                c_a = per.tile(E3, f32, tag="c_a")
                nc.scalar.activation(
                    out=c_a[:, :, :], in_=d_t[:, :, :], func=ACTF.Sin,
                    bias=pi2, scale=float(-np.pi / (2.0 * ANG_CUT)))
                nc.scalar.activation(
                    out=c_a[:, :, :], in_=c_a[:, :, :], func=ACTF.Relu)
                w_t = per.tile(E3, f32, tag="w")
                nc.scalar.activation(
                    out=w_t[:, :, :], in_=c_a[:, :, :], func=ACTF.Square)

                # ---- radial chain: P_c = T_c(xr) * fc_rad ----
                p_prev2 = chebp.tile(E3, f32, tag="pch")
                nc.vector.tensor_scalar(
                    out=p_prev2[:, :, :], in0=c_r[:, :, :],
                    scalar1=0.5, scalar2=0.5, op0=ALU.mult, op1=ALU.add)
                p_prev1 = chebp.tile(E3, f32, tag="pch")
                nc.vector.scalar_tensor_tensor(
                    out=p_prev1[:, :, :], in0=xr2[:, :, :], scalar=0.5,
                    in1=p_prev2[:, :, :], op0=ALU.mult, op1=ALU.mult)

                def rad_out(ptile, c):
                    nc.vector.tensor_reduce(
                        out=out_t[:, :, c], in_=ptile[:, :, :],
                        axis=AX.X, op=ALU.add)
                    gw = rot.tile(E3, f32, tag="scr")
                    nc.vector.tensor_mul(gw[:, :, :], ptile[:, :, :], ts_f[:, :, :])
                    nc.vector.tensor_reduce(
                        out=out_t[:, :, NRAD + c], in_=gw[:, :, :],
                        axis=AX.X, op=ALU.add)

                rad_out(p_prev2, 0)
                rad_out(p_prev1, 1)
                for c in range(2, NRAD):
                    m = rot.tile(E3, f32, tag="scr")
                    nc.vector.tensor_mul(m[:, :, :], xr2[:, :, :], p_prev1[:, :, :])
                    p_cur = chebp.tile(E3, f32, tag="pch")
                    nc.vector.tensor_sub(p_cur[:, :, :], m[:, :, :], p_prev2[:, :, :])
                    rad_out(p_cur, c)
                    p_prev2, p_prev1 = p_prev1, p_cur

                # ---- angular ----
                ws_t = per.tile(E3, f32, tag="ws")
                nc.vector.tensor_mul(ws_t[:, :, :], w_t[:, :, :], ts_f[:, :, :])

                # D = sum w^2 per atom; Dh = D/2 broadcast over the set dim
                wsq = rot.tile(E3, f32, tag="scr")
                nc.scalar.activation(
                    out=wsq[:, :, :], in_=w_t[:, :, :], func=ACTF.Square)
                dh = per.tile([128, A_PART, 1], f32, tag="dh")
                nc.vector.tensor_reduce(
                    out=dh[:, :, 0], in_=wsq[:, :, :], axis=AX.X, op=ALU.add)
                nc.vector.tensor_scalar_mul(dh[:, :, :], dh[:, :, :], 0.5)
                dhb = dh[:, :, :].to_broadcast((128, A_PART, 2))

                ux = u_t[:, :, :, 0]
                uy = u_t[:, :, :, 1]
                uz = u_t[:, :, :, 2]

                # mom[:, a, s, ch]: s=0 plain weights w, s=1 typespin ws
                mom = per.tile([128, A_PART, 2, 20], f32, tag="mom")

                def moments(wgt, s):
                    def red(src, ch):
                        nc.vector.tensor_reduce(
                            out=mom[:, :, s, ch], in_=src, axis=AX.X, op=ALU.add)

                    red(wgt[:, :, :], 0)
                    p = {}
                    for i, (nm, uc) in enumerate((("x", ux), ("y", uy), ("z", uz))):
                        pt = shared.tile(E3, f32, tag=f"p{i}")
                        nc.vector.tensor_mul(pt[:, :, :], wgt[:, :, :], uc)
                        red(pt[:, :, :], 1 + i)
                        p[nm] = pt
                    q = {}
                    for nm, (pa, uc, ch) in {
                            "xx": ("x", ux, 4), "yy": ("y", uy, 5),
                            "xy": ("x", uy, 7), "xz": ("x", uz, 8),
                            "yz": ("y", uz, 9)}.items():
                        qt = shared.tile(E3, f32, tag=f"q{nm}")
                        nc.vector.tensor_mul(qt[:, :, :], p[pa][:, :, :], uc)
                        red(qt[:, :, :], ch)
                        q[nm] = qt
                    # M3 order: xxx,xxy,xxz,xyy,yyy,yyz,xyz (xzz,yzz,zzz derived)
                    for qk, uc, ch in (
                            ("xx", ux, 10), ("xx", uy, 11), ("xx", uz, 12),
                            ("yy", ux, 13), ("yy", uy, 14), ("yy", uz, 15),
                            ("xy", uz, 16)):
                        rt = rot.tile(E3, f32, tag="scr")
                        nc.vector.tensor_mul(rt[:, :, :], q[qk][:, :, :], uc)
                        red(rt[:, :, :], ch)
                    # derived via |u|=1:
                    #   zz = M0 - xx - yy
                    t1 = shared.tile([128, A_PART, 1], f32, tag="dt1")
                    nc.vector.tensor_add(
                        t1[:, :, :], mom[:, :, s, 4:5], mom[:, :, s, 5:6])
                    nc.vector.tensor_sub(
                        mom[:, :, s, 6:7], mom[:, :, s, 0:1], t1[:, :, :])
                    #   (xzz,yzz,zzz) = M1 - (xxx,xxy,xxz) - (xyy,yyy,yyz)
                    t3 = shared.tile([128, A_PART, 3], f32, tag="dt3")
                    nc.vector.tensor_add(
                        t3[:, :, :], mom[:, :, s, 10:13], mom[:, :, s, 13:16])
                    nc.vector.tensor_sub(
                        mom[:, :, s, 17:20], mom[:, :, s, 1:4], t3[:, :, :])

                moments(w_t, 0)
                moments(ws_t, 1)

                if nocombine:
                    nc.vector.tensor_copy(
                        out=out_t[:, :, 2 * NRAD:NOUT].rearrange(
                            "p a (s c) -> p a s c", s=2),
                        in_=mom[:, :, :, 0:4])
                    for lo, hi in ((0, 43), (43, 86), (86, 128)):
                        nc.sync.dma_start(out=out_dram.ap()[lo:hi], in_=out_t[lo:hi])
                    return

                # ---- combine, both sets at once ----
                sq = per.tile([128, A_PART, 2, 20], f32, tag="sq")
                nc.vector.tensor_mul(
                    sq[:, :, :, :], mom[:, :, :, :], mom[:, :, :, :])
                for s in (0, 1):
                    nc.vector.tensor_mul(
                        sq[:, :, s, :], sq[:, :, s, :],
                        wvec[:, :, :].to_broadcast((128, A_PART, 20)))
                sv = per.tile([128, A_PART, 2, 3], f32, tag="sv")  # S1,C2,C3
                for s in (0, 1):
                    nc.vector.tensor_reduce(
                        out=sv[:, :, s, 0], in_=sq[:, :, s, 1:4],
                        axis=AX.X, op=ALU.add)
                    nc.vector.tensor_reduce(
                        out=sv[:, :, s, 1], in_=sq[:, :, s, 4:10],
                        axis=AX.X, op=ALU.add)
                    nc.vector.tensor_reduce(
                        out=sv[:, :, s, 2], in_=sq[:, :, s, 10:20],
                        axis=AX.X, op=ALU.add)
                s0 = sq[:, :, :, 0]
                s1 = sv[:, :, :, 0]
                c2 = sv[:, :, :, 1]
                c3 = sv[:, :, :, 2]
                # oa[:, a, s, c]: c=0..3 angular outputs for set s
                oa = per.tile([128, A_PART, 2, 4], f32, tag="oa")
                nc.vector.scalar_tensor_tensor(
                    out=oa[:, :, :, 0], in0=s0, scalar=0.5,
                    in1=dhb, op0=ALU.mult, op1=ALU.subtract)
                nc.vector.scalar_tensor_tensor(
                    out=oa[:, :, :, 1], in0=s1, scalar=0.5,
                    in1=dhb, op0=ALU.mult, op1=ALU.subtract)
                t1 = per.tile([128, A_PART, 2], f32, tag="ct1")
                nc.vector.scalar_tensor_tensor(
                    out=t1[:, :, :], in0=s0, scalar=0.5,
                    in1=dhb, op0=ALU.mult, op1=ALU.add)
                nc.vector.tensor_sub(oa[:, :, :, 2], c2, t1[:, :, :])
                t2 = per.tile([128, A_PART, 2], f32, tag="ct2")
                nc.vector.scalar_tensor_tensor(
                    out=t2[:, :, :], in0=s1, scalar=1.5,
                    in1=dhb, op0=ALU.mult, op1=ALU.add)
                nc.vector.scalar_tensor_tensor(
                    out=oa[:, :, :, 3], in0=c3, scalar=2.0,
                    in1=t2[:, :, :], op0=ALU.mult, op1=ALU.subtract)
                # oa [A,2,4] flattens to exactly out channels 22..30
                nc.vector.tensor_copy(
                    out=out_t[:, :, 2 * NRAD:NOUT],
                    in_=oa[:, :, :, :].rearrange("p a s c -> p a (s c)"))

                for lo, hi in ((0, 43), (43, 86), (86, 128)):
                    nc.sync.dma_start(out=out_dram.ap()[lo:hi], in_=out_t[lo:hi])

            if loop_n == 1:
                body()
            else:
                with tc.For_i(0, loop_n, 1) as iv:
                    body(iv)

    nc.compile()
    return nc


def _get_compiled(loop_n: int = 1):
    if loop_n not in _COMPILED:
        _COMPILED[loop_n] = build_program(loop_n)
    return _COMPILED[loop_n]


def _make_in_maps(distances, unit_vecs, neighbor_species):
    d = np.ascontiguousarray(np.asarray(distances, dtype=np.float32))
    u = np.ascontiguousarray(np.asarray(unit_vecs, dtype=np.float32))
    sp = np.ascontiguousarray(np.asarray(neighbor_species, dtype=np.int32))
    E = N_ATOMS * K
    EP = NPAD * K
    dp = np.zeros(EP, np.float32)
    dp[:E] = d
    up = np.zeros((EP, 3), np.float32)
    up[:E] = u
    spp = np.zeros(EP, np.int32)
    spp[:E] = sp
    in_maps = []
    ec = NA_CORE * K
    for c in range(NCORES):
        s = slice(c * ec, (c + 1) * ec)
        in_maps.append({
            "d": dp[s].reshape(128, A_PART, K),
            "u": up[s].reshape(128, A_PART, K, 3),
            "sp": spp[s].reshape(128, A_PART, K),
        })
    return in_maps


def run_on_hw(in_maps, loop_n: int = 1):
    from concourse.bass_utils import run_bass_kernel_spmd
    nc = _get_compiled(loop_n)
    return run_bass_kernel_spmd(nc, in_maps, core_ids=list(range(NCORES)))


def kernel(distances, unit_vecs, center_idx=None, neighbor_species=None,
           triplet_center=None, triplet_j=None, triplet_k=None,
           n_atoms=N_ATOMS, **_unused):
    in_maps = _make_in_maps(distances, unit_vecs, neighbor_species)
    res = run_on_hw(in_maps, loop_n=1)
    out = np.concatenate(
        [r["out"].reshape(NA_CORE, NOUT) for r in res.results], axis=0)
    return np.ascontiguousarray(out[:N_ATOMS])


if __name__ == "__main__":
    rng = np.random.default_rng(0)
    E = N_ATOMS * K
    d = rng.uniform(MIN_CUT + 0.05, RAD_CUT, size=E).astype(np.float32)
    v = rng.normal(size=(E, 3))
    u = (v / np.linalg.norm(v, axis=1, keepdims=True)).astype(np.float32)
    sp = rng.integers(0, 2, size=E).astype(np.int32)
    out = kernel(d, u, neighbor_species=sp)
    print(out.shape, out.dtype, out[:2])



# revision 8
# speedup vs baseline: 3.3484x; 3.3484x over previous
"""Chebyshev atomic descriptor kernel for 8 Trainium2 NeuronCores.

Math (matches the jax reference up to fp reassociation):
  radial: P_c = T_c(xr)*fc_rad per edge via the Chebyshev recurrence,
    summed per atom, plain and typespin-weighted (fp16 per-edge tensors).
  angular: all-pairs (j<k) sums reduce to symmetric moment tensors
    M0..M3 of the weighted unit vectors; trace identities (|u|=1) derive
    zz/xzz/yzz/zzz rows inside the reduction stationaries; the final
    angular outputs are a fixed linear map of the squared moments and
    D = sum w^2, evaluated by a second tensor-engine pass.

Layout: per core 6272 atoms as [120 partitions = 5 atoms x 24 edges,
  1256 free = atom groups]; atom a = 5*f + am. Per-atom K-sums are
  tensor-engine matmuls with ones-block stationaries; 23/20/20-channel
  PSUM packs keep drains amortized. DVE does the ~61 fp16 elementwise
  muls; ScalarE does all unary ops and PSUM drains.
"""

import numpy as np

N_ATOMS = 50000
K = 24
RAD_ORDER = 10
RAD_CUT = 8.0
ANG_CUT = 6.5
MIN_CUT = 0.55
NCORES = 8
AM = 5                      # atoms per partition-group
PPART = AM * K              # 120 partitions used
FD = 1256                   # atom-group columns per core
NA_CORE = AM * FD           # 6280 atom slots per core (6272 used)
NRAD = RAD_ORDER + 1        # 11
NOUT = 2 * (NRAD + 4)       # 30

# channel indices within the GB/GC (angular) 20-row packs
# computed movings (16): w, x, y, z, xx, yy, xy, xz, yz,
#                        xxx, xxy, xxz, xyy, yyy, yyz, xyz
# pack rows (20): 0:w 1:x 2:y 3:z 4:xx 5:yy 6:zz* 7:xy 8:xz 9:yz
#                 10:xxx 11:xxy 12:xxz 13:xyy 14:yyy 15:yyz 16:xyz
#                 17:xzz* 18:yzz* 19:zzz*   (* derived via stationary)
ANG_STAT_BLOCKS = [
    # moving index -> list of (pack_row, sign)
    [(0, 1.0), (6, 1.0)],            # w      (+zz row: M0)
    [(1, 1.0), (17, 1.0)],           # x      (+xzz: M1x)
    [(2, 1.0), (18, 1.0)],           # y      (+yzz: M1y)
    [(3, 1.0), (19, 1.0)],           # z      (+zzz: M1z)
    [(4, 1.0), (6, -1.0)],           # xx     (-zz)
    [(5, 1.0), (6, -1.0)],           # yy     (-zz)
    [(7, 1.0)],                      # xy
    [(8, 1.0)],                      # xz
    [(9, 1.0)],                      # yz
    [(10, 1.0), (17, -1.0)],         # xxx    (-xzz)
    [(11, 1.0), (18, -1.0)],         # xxy    (-yzz)
    [(12, 1.0), (19, -1.0)],         # xxz    (-zzz)
    [(13, 1.0), (17, -1.0)],         # xyy    (-xzz)
    [(14, 1.0), (18, -1.0)],         # yyy    (-yzz)
    [(15, 1.0), (19, -1.0)],         # yyz    (-zzz)
    [(16, 1.0)],                     # xyz
]

# PE2 linear map: angular outputs oa[m] from squared pack rows q[0..19]
# and D:  oa0 = .5 q0 - .5 D ; oa1 = .5 (q1+q2+q3) - .5 D
# oa2 = (q4+q5+q6) + 2(q7+q8+q9) - .5 q0 - .5 D
# oa3 = 2*(q10+3q11+3q12+3q13+q14+3q15+6q16+3q17+3q18+q19)
#       - 1.5(q1+q2+q3) - .5 D
W3 = [1.0, 3.0, 3.0, 3.0, 1.0, 3.0, 6.0, 3.0, 3.0, 1.0]


def _pe2_coeffs():
    co = np.zeros((4, 20), np.float32)
    co[0, 0] = 0.5
    co[1, 1:4] = 0.5
    co[2, 0] = -0.5
    co[2, 4:7] = 1.0
    co[2, 7:10] = 2.0
    co[3, 1:4] = -1.5
    for j, wgt in enumerate(W3):
        co[3, 10 + j] = 2.0 * wgt
    return co


_COMPILED = {}
_CONSTS = {}


def _host_consts():
    """Constant stationary buffers shipped to every core."""
    if _CONSTS:
        return _CONSTS
    # GA sliding-window buffer: [120, 225] with ones-block at cols 110..114
    ga = np.zeros((PPART, 2 * 110 + 5), np.float16)
    for am in range(AM):
        ga[am * K:(am + 1) * K, 110 + am] = 1.0
    # GB stationaries: 16 variants of [120, 100]
    gb = np.zeros((PPART, 16 * 100), np.float16)
    for mv, blocks in enumerate(ANG_STAT_BLOCKS):
        for row, sign in blocks:
            for am in range(AM):
                gb[am * K:(am + 1) * K, mv * 100 + row * AM + am] = sign
    # PE2 stationaries: over squared pack rows [100, 40] for set0 / set1
    co = _pe2_coeffs()
    # p2 [100, 80]: cols 0..39 = set0 variant (writes rows 0..19),
    # cols 40..79 = set1 variant (writes rows 20..39)
    p2 = np.zeros((100, 80), np.float16)
    for s in range(2):
        for m in range(4):
            for ch in range(20):
                for am in range(AM):
                    p2[ch * AM + am, s * 40 + s * 20 + m * AM + am] = co[m, ch]
    # D stationary [120, 40]: edge-level w^2 reduce with -0.5, both sets
    pd = np.zeros((PPART, 40), np.float16)
    for s in range(2):
        for m in range(4):
            for am in range(AM):
                pd[am * K:(am + 1) * K, s * 20 + m * AM + am] = -0.5
    _CONSTS.update(ga=ga, gb=gb, p2=p2, pd=pd)
    return _CONSTS


def build_program(loop_n: int = 1):
    import concourse.bacc as bacc
    import concourse.mybir as mybir
    from concourse.tile import TileContext

    f32 = mybir.dt.float32
    f16 = mybir.dt.float16
    ALU = mybir.AluOpType
    ACTF = mybir.ActivationFunctionType

    nc = bacc.Bacc("TRN2", target_bir_lowering=False)

    pi2 = float(np.pi / 2)
    _cst = nc.alloc_sbuf_tensor("const-float32-pi2", [128, 1], f32)
    nc.gpsimd.memset(_cst.ap(), pi2)
    nc.const_aps.aps[(f32, pi2)] = _cst.ap()
    nc.all_engine_barrier()

    d_dram = nc.dram_tensor("d", [PPART, FD], f32, kind="ExternalInput")
    ux_dram = nc.dram_tensor("ux", [PPART, FD], f16, kind="ExternalInput")
    uy_dram = nc.dram_tensor("uy", [PPART, FD], f16, kind="ExternalInput")
    uz_dram = nc.dram_tensor("uz", [PPART, FD], f16, kind="ExternalInput")
    ts_dram = nc.dram_tensor("ts", [PPART, FD], f16, kind="ExternalInput")
    ga_dram = nc.dram_tensor("gast", [PPART, 225], f16, kind="ExternalInput")
    gb_dram = nc.dram_tensor("gbst", [PPART, 1600], f16, kind="ExternalInput")
    p2_dram = nc.dram_tensor("p2st", [100, 80], f16, kind="ExternalInput")
    pd_dram = nc.dram_tensor("pdst", [PPART, 40], f16, kind="ExternalInput")
    rad_dram = nc.dram_tensor("rad", [110, FD], f32, kind="ExternalOutput")
    ang_dram = nc.dram_tensor("ang", [40, FD], f32, kind="ExternalOutput")

    ax = 2.0 / (RAD_CUT - MIN_CUT)
    bx = -MIN_CUT * ax - 1.0
    CH = [0, 512, 1024, FD]  # phase chunk boundaries

    with TileContext(nc) as tc:
        with (
            tc.tile_pool(name="inp", bufs=1) as inp,
            tc.tile_pool(name="mov", bufs=1) as mov,
            tc.tile_pool(name="outp", bufs=1) as outp,
            tc.tile_pool(name="scr", bufs=4) as scr,
            tc.psum_pool(name="ps", bufs=2) as psp,
        ):
            d32 = inp.tile([PPART, FD], f32, tag="d32")
            ux = inp.tile([PPART, FD], f16, tag="ux")
            uy = inp.tile([PPART, FD], f16, tag="uy")
            uz = inp.tile([PPART, FD], f16, tag="uz")
            ts = inp.tile([PPART, FD], f16, tag="ts")
            gast = inp.tile([PPART, 225], f16, tag="gast")
            gbst = inp.tile([PPART, 1600], f16, tag="gbst")
            p2st = inp.tile([100, 80], f16, tag="p2st")
            pdst = inp.tile([PPART, 40], f16, tag="pdst")

            def loads():
                # d first (unblocks the ScalarE chain), then the rest;
                # stationaries issue from the Scalar queue (idle until d).
                nc.sync.dma_start(out=d32[:, :], in_=d_dram.ap())
                nc.sync.dma_start(out=ts[:, :], in_=ts_dram.ap())
                nc.sync.dma_start(out=ux[:, :], in_=ux_dram.ap())
                nc.sync.dma_start(out=uy[:, :], in_=uy_dram.ap())
                nc.sync.dma_start(out=uz[:, :], in_=uz_dram.ap())
                nc.sync.dma_start(out=gast[:, :], in_=ga_dram.ap())
                nc.sync.dma_start(out=gbst[:, :], in_=gb_dram.ap())
                nc.sync.dma_start(out=p2st[:, :], in_=p2_dram.ap())
                nc.sync.dma_start(out=pdst[:, :], in_=pd_dram.ap())

            def body(_iv=None):
                loads()

                # ---- ScalarE unary chain (Sin ops first: one table ctx) ----
                s_r = scr.tile([PPART, FD], f16, tag="scr")
                nc.scalar.activation(out=s_r[:, :], in_=d32[:, :], func=ACTF.Sin,
                                     bias=pi2, scale=float(-np.pi / RAD_CUT))
                p0 = mov.tile([PPART, FD], f16, tag="p0")
                nc.scalar.activation(out=p0[:, :], in_=s_r[:, :],
                                     func=ACTF.Copy, bias=0.5, scale=0.5)
                xr = mov.tile([PPART, FD], f16, tag="xr")
                nc.scalar.activation(out=xr[:, :], in_=d32[:, :],
                                     func=ACTF.Copy, bias=bx, scale=ax)
                xr2 = mov.tile([PPART, FD], f16, tag="xr2")
                nc.scalar.activation(out=xr2[:, :], in_=d32[:, :],
                                     func=ACTF.Copy, bias=2 * bx, scale=2 * ax)
                s_a = scr.tile([PPART, FD], f16, tag="scr")
                nc.scalar.activation(out=s_a[:, :], in_=d32[:, :], func=ACTF.Sin,
                                     bias=pi2, scale=float(-np.pi / (2 * ANG_CUT)))
                r_a = scr.tile([PPART, FD], f16, tag="scr")
                nc.scalar.activation(out=r_a[:, :], in_=s_a[:, :], func=ACTF.Relu)
                w = mov.tile([PPART, FD], f16, tag="w")
                nc.scalar.activation(out=w[:, :], in_=r_a[:, :], func=ACTF.Square)
                w2e = mov.tile([PPART, FD], f16, tag="w2e")
                nc.scalar.activation(out=w2e[:, :], in_=w[:, :], func=ACTF.Square)

                # ---- DVE muls: radial chain ----
                radm = [p0]
                p1 = mov.tile([PPART, FD], f16, tag="p1")
                nc.vector.tensor_mul(p1[:, :], xr[:, :], p0[:, :])
                radm.append(p1)
                prev2, prev1 = p0, p1
                for c in range(2, NRAD):
                    t = scr.tile([PPART, FD], f16, name=f"t{c}", tag="scr")
                    nc.vector.tensor_mul(t[:, :], xr2[:, :], prev1[:, :])
                    pc = mov.tile([PPART, FD], f16, name=f"pc{c}", tag=f"pc{c}")
                    nc.vector.tensor_sub(pc[:, :], t[:, :], prev2[:, :])
                    radm.append(pc)
                    prev2, prev1 = prev1, pc

                # ---- DVE muls: angular chains ----
                def ang_chain(base):
                    out = [base]
                    nm = base.tensor.name[:2]
                    px = mov.tile([PPART, FD], f16, name=f"{nm}px", tag=f"{nm}px")
                    nc.vector.tensor_mul(px[:, :], base[:, :], ux[:, :])
                    py = mov.tile([PPART, FD], f16, name=f"{nm}py", tag=f"{nm}py")
                    nc.vector.tensor_mul(py[:, :], base[:, :], uy[:, :])
                    pz = mov.tile([PPART, FD], f16, name=f"{nm}pz", tag=f"{nm}pz")
                    nc.vector.tensor_mul(pz[:, :], base[:, :], uz[:, :])
                    out += [px, py, pz]
                    qxx = mov.tile([PPART, FD], f16, name=f"{nm}qxx", tag=f"{nm}qxx")
                    nc.vector.tensor_mul(qxx[:, :], px[:, :], ux[:, :])
                    qyy = mov.tile([PPART, FD], f16, name=f"{nm}qyy", tag=f"{nm}qyy")
                    nc.vector.tensor_mul(qyy[:, :], py[:, :], uy[:, :])
                    qxy = mov.tile([PPART, FD], f16, name=f"{nm}qxy", tag=f"{nm}qxy")
                    nc.vector.tensor_mul(qxy[:, :], px[:, :], uy[:, :])
                    qxz = mov.tile([PPART, FD], f16, name=f"{nm}qxz", tag=f"{nm}qxz")
                    nc.vector.tensor_mul(qxz[:, :], px[:, :], uz[:, :])
                    qyz = mov.tile([PPART, FD], f16, name=f"{nm}qyz", tag=f"{nm}qyz")
                    nc.vector.tensor_mul(qyz[:, :], py[:, :], uz[:, :])
                    out += [qxx, qyy, qxy, qxz, qyz]
                    for src, uc, lbl in ((qxx, ux, "xxx"), (qxx, uy, "xxy"),
                                         (qxx, uz, "xxz"), (qyy, ux, "xyy"),
                                         (qyy, uy, "yyy"), (qyy, uz, "yyz"),
                                         (qxy, uz, "xyz")):
                        cc = mov.tile([PPART, FD], f16, name=f"{nm}{lbl}",
                                      tag=f"{nm}{lbl}")
                        nc.vector.tensor_mul(cc[:, :], src[:, :], uc[:, :])
                        out.append(cc)
                    return out

                # typespin-weighted radial
                for c in range(NRAD):
                    qc = mov.tile([PPART, FD], f16, name=f"qc{c}", tag=f"qc{c}")
                    nc.vector.tensor_mul(qc[:, :], radm[c][:, :], ts[:, :])
                    radm.append(qc)

                gbm = ang_chain(w)
                ws = mov.tile([PPART, FD], f16, tag="ws")
                nc.vector.tensor_mul(ws[:, :], w[:, :], ts[:, :])
                gcm = ang_chain(ws)

                # ---- PE1 reductions, phase-outer / channel-inner ----
                rad_out = outp.tile([110, FD], f32, tag="rad_out")
                sqb = outp.tile([100, FD], f16, tag="sqb")
                sqc = outp.tile([100, FD], f16, tag="sqc")
                ang_out = outp.tile([40, FD], f32, tag="ang_out")

                for ph in range(3):
                    lo, hi = CH[ph], CH[ph + 1]
                    wd = hi - lo
                    ga_ps = psp.tile([110, wd], f32, name=f"ga{ph}", tag="gaps")
                    for c, m in enumerate(radm):
                        nc.tensor.matmul(
                            out=ga_ps[:, :],
                            lhsT=gast[:, 110 - 5 * c:220 - 5 * c],
                            rhs=m[:, lo:hi],
                            start=(c == 0), stop=(c == len(radm) - 1))
                    gb_ps = psp.tile([100, wd], f32, name=f"gb{ph}", tag="gbps")
                    for c, m in enumerate(gbm):
                        nc.tensor.matmul(
                            out=gb_ps[:, :],
                            lhsT=gbst[:, c * 100:(c + 1) * 100],
                            rhs=m[:, lo:hi],
                            start=(c == 0), stop=(c == 15))
                    gc_ps = psp.tile([100, wd], f32, name=f"gc{ph}", tag="gcps")
                    for c, m in enumerate(gcm):
                        nc.tensor.matmul(
                            out=gc_ps[:, :],
                            lhsT=gbst[:, c * 100:(c + 1) * 100],
                            rhs=m[:, lo:hi],
                            start=(c == 0), stop=(c == 15))

                    # drains
                    nc.scalar.activation(out=rad_out[:, lo:hi], in_=ga_ps[:, :],
                                         func=ACTF.Copy)
                    nc.scalar.activation(out=sqb[:, lo:hi], in_=gb_ps[:, :],
                                         func=ACTF.Square)
                    nc.scalar.activation(out=sqc[:, lo:hi], in_=gc_ps[:, :],
                                         func=ACTF.Square)
                    # ---- PE2: angular combine ----
                    p2_ps = psp.tile([40, wd], f32, name=f"p2{ph}", tag="p2ps")
                    nc.tensor.matmul(out=p2_ps[:, :], lhsT=p2st[:, 0:40],
                                     rhs=sqb[:, lo:hi], start=True, stop=False)
                    nc.tensor.matmul(out=p2_ps[:, :], lhsT=p2st[:, 40:80],
                                     rhs=sqc[:, lo:hi], start=False, stop=False)
                    nc.tensor.matmul(out=p2_ps[:, :], lhsT=pdst[:, 0:40],
                                     rhs=w2e[:, lo:hi], start=False, stop=True)
                    nc.scalar.activation(out=ang_out[:, lo:hi], in_=p2_ps[:, :],
                                         func=ACTF.Copy)
                    # stream outputs per phase
                    nc.sync.dma_start(out=rad_dram.ap()[:, lo:hi],
                                      in_=rad_out[:, lo:hi])
                    nc.sync.dma_start(out=ang_dram.ap()[:, lo:hi],
                                      in_=ang_out[:, lo:hi])

            if loop_n == 1:
                body()
            else:
                with tc.For_i(0, loop_n, 1) as iv:
                    body(iv)

    nc.compile()
    return nc


def _get_compiled(loop_n: int = 1):
    if loop_n not in _COMPILED:
        _COMPILED[loop_n] = build_program(loop_n)
    return _COMPILED[loop_n]


def _make_in_maps(distances, unit_vecs, neighbor_species):
    d = np.ascontiguousarray(np.asarray(distances, dtype=np.float32))
    u = np.ascontiguousarray(np.asarray(unit_vecs, dtype=np.float32))
    sp = np.ascontiguousarray(np.asarray(neighbor_species, dtype=np.int32))
    E = N_ATOMS * K
    NPAD = NCORES * NA_CORE
    EP = NPAD * K
    # pad with dead edges: d=8 -> fc_rad=0 and fc_ang=0 exactly
    dp = np.full(EP, 8.0, np.float32)
    dp[:E] = d
    up = np.zeros((EP, 3), np.float16)
    up[:E] = u.astype(np.float16)
    tp = np.ones(EP, np.float16)
    tp[:E] = (2 * sp - 1).astype(np.float16)
    cst = _host_consts()
    in_maps = []
    for c in range(NCORES):
        s = slice(c * NA_CORE * K, (c + 1) * NA_CORE * K)
        # atoms [NA_CORE, K] -> [FD groups, AM, K] -> partitions (am, k)
        dd = dp[s].reshape(FD, AM, K).transpose(1, 2, 0).reshape(PPART, FD)
        uu = up[s].reshape(FD, AM, K, 3).transpose(3, 1, 2, 0)
        tt = tp[s].reshape(FD, AM, K).transpose(1, 2, 0).reshape(PPART, FD)
        in_maps.append({
            "d": np.ascontiguousarray(dd),
            "ux": np.ascontiguousarray(uu[0].reshape(PPART, FD)),
            "uy": np.ascontiguousarray(uu[1].reshape(PPART, FD)),
            "uz": np.ascontiguousarray(uu[2].reshape(PPART, FD)),
            "ts": np.ascontiguousarray(tt),
            "gast": cst["ga"], "gbst": cst["gb"],
            "p2st": cst["p2"], "pdst": cst["pd"],
        })
    return in_maps


def run_on_hw(in_maps, loop_n: int = 1):
    from concourse.bass_utils import run_bass_kernel_spmd
    nc = _get_compiled(loop_n)
    return run_bass_kernel_spmd(nc, in_maps, core_ids=list(range(NCORES)))


def kernel(distances, unit_vecs, center_idx=None, neighbor_species=None,
           triplet_center=None, triplet_j=None, triplet_k=None,
           n_atoms=N_ATOMS, **_unused):
    in_maps = _make_in_maps(distances, unit_vecs, neighbor_species)
    res = run_on_hw(in_maps, loop_n=1)
    out = np.empty((NCORES * NA_CORE, NOUT), np.float32)
    for c, r in enumerate(res.results):
        # rad rows (ch, am): ch 0..21 -> out cols 0..21 ; row 22 = D (skip)
        rad = r["rad"].reshape(22, AM, FD)
        ang = r["ang"].reshape(2, 4, AM, FD)
        o = out[c * NA_CORE:(c + 1) * NA_CORE].reshape(FD, AM, NOUT)
        o[:, :, 0:22] = rad[0:22].transpose(2, 1, 0)
        o[:, :, 22:26] = ang[0].transpose(2, 1, 0)
        o[:, :, 26:30] = ang[1].transpose(2, 1, 0)
    return np.ascontiguousarray(out[:N_ATOMS])


if __name__ == "__main__":
    rng = np.random.default_rng(0)
    E = N_ATOMS * K
    d = rng.uniform(MIN_CUT + 0.05, RAD_CUT, size=E).astype(np.float32)
    v = rng.normal(size=(E, 3))
    u = (v / np.linalg.norm(v, axis=1, keepdims=True)).astype(np.float32)
    sp = rng.integers(0, 2, size=E).astype(np.int32)
    out = kernel(d, u, neighbor_species=sp)
    print(out.shape, out.dtype, out[:2])
